# revision 36
# baseline (speedup 1.0000x reference)
"""GATv2 x3 + MLP (nn_GAT) on trn2, 8 NeuronCores.

Graph-parallel: host shards edges by dst-node range (25K nodes/core),
degree-sorts nodes into a padded-ELL schedule, and lays out per-edge
SOURCE features as packed sequential streams (layout/replication of
input bytes only -- every FLOP runs on device).

v2 rewrite vs baseline:
 - Swapped-operand projection: the packed edge stream block is the
   matmul's stationary operand, the tiny block-diag weight streams
   -> PSUM comes out node-major directly. No PE transposes, one
   PSUM->SBUF copy per 2-10 blocks (was 2 copies + 1 transpose per
   128 cols; ~300us of ACT/DVE saved).
 - 16/12 slots per packed column (K=80/120) halves LDWEIGHTS count.
 - Biases folded into the dst-side local projection -> edge streams
   carry only raw features (40->80/120 rows, no const row).
 - GAT pipeline in-place: e=G+fd (in G), w=e*ex (in G); the fs-vs-e
   difference is corrected per-node in a batched epilogue using
   rstn_true = rstn_e - fd * den_raw  (pad slots cancel exactly).
 - l-reduction via tensor_tensor tree-adds (DVE 2x) instead of 1x
   tensor_reduce; score/den reductions offloaded to the idle GpSimd
   engine; softmax denominators pad-corrected per-node in the epilogue.
 - MLP: single [128,56] transpose per 512-node chunk; sigmoid computed
   as exp (same ACT table as the GAT exps -> no ACT_TABLE_LOAD thrash)
   + 1/(1+y) on DVE with fast reciprocal.
"""
import sys
sys.path.insert(0, '/opt/trn_rl_repo')
import numpy as np
import ml_dtypes

import concourse.bass as bass
import concourse.mybir as mybir
from concourse import bacc
from concourse.tile import TileContext
from concourse.bass_utils import run_bass_kernel_spmd
from concourse.masks import make_identity

bf16 = mybir.dt.bfloat16
f32 = mybir.dt.float32
BF = ml_dtypes.bfloat16
AL = mybir.AluOpType
AF = mybir.ActivationFunctionType
AX = mybir.AxisListType

NCORE = 8
P = 128
B = 8
A1 = 16   # slots multiplexed per packed column, launch 1 (K = 16*5 = 80)
A2 = 12   # launch 2 (K = 12*10 = 120)
NEG_GAT = 0.2
NEG_MLP = 0.01


# ================================================================= host prep
def build_schedule(dst, n):
    nloc = n // NCORE
    core_of = dst // nloc
    scheds = []
    for c in range(NCORE):
        em = np.where(core_of == c)[0]
        ldst = dst[em] - c * nloc
        deg = np.bincount(ldst, minlength=nloc)
        nt = -(-nloc // P)
        nt = -(-nt // B) * B
        degp = np.concatenate([deg, np.zeros(nt * P - nloc, np.int64)])
        order = np.argsort(-degp, kind='stable')
        pos_of = np.empty_like(order)
        pos_of[order] = np.arange(len(order))
        scheds.append(dict(core=c, em=em, ldst=ldst, deg=degp, order=order,
                           pos_of=pos_of, nt=nt, nloc=nloc))
    nt = scheds[0]['nt']
    nst = nt // B
    Ls = []
    for st in range(nst):
        L = 1
        for s in scheds:
            L = max(L, int(s['deg'][s['order'][st * B * P]]))
        Ls.append(L)
    return scheds, nst, Ls


def edge_slot_cols(s, Ls, A):
    nblk = [-(-(B * L) // A) for L in Ls]
    offs = np.concatenate([[0], np.cumsum([nb * P for nb in nblk])]).astype(np.int64)
    order, deg = s['order'], s['deg']
    pos_e = s['pos_of'][s['ldst']]
    eo = np.lexsort((np.arange(len(pos_e)), pos_e))
    pos_sorted = pos_e[eo]
    starts = np.concatenate([[0], np.cumsum(deg[order])])
    rank = np.arange(len(eo)) - starts[pos_sorted]
    st_of = (pos_sorted // P) // B
    L_e = np.asarray(Ls)[st_of]
    q_e = ((pos_sorted // P) % B) * L_e + rank
    col_e = (offs[st_of] + (q_e // A) * P + (pos_sorted % P)).astype(np.int64)
    a_e = (q_e % A).astype(np.int64)
    return eo, a_e, col_e, nblk, offs


def make_npad(s, Ls, rep):
    deg, order = s['deg'], s['order']
    nt = s['nt']
    npad = np.zeros((P, nt), np.float32)
    for t in range(nt):
        L = Ls[t // B]
        npad[:, t] = L - deg[order[t * P:(t + 1) * P]]
    return np.repeat(npad[:, :, None], rep, axis=2).reshape(P, nt * rep).astype(BF)


def pack_edges(feats, eo, a_e, col_e, totc, A):
    nf = feats.shape[1]
    pk = np.zeros((A * nf, totc), BF)
    fe = feats[eo].astype(BF)
    for f in range(nf):
        pk[a_e * nf + f, col_e] = fe[:, f]
    return pk


def pack_local(vals, nrow, nt):
    pk = np.zeros((8 * nrow, (nt // 8) * P), BF)
    nodes = np.arange(nt * P)
    a = (nodes // P) % 8
    col = (nodes // (8 * P)) * P + nodes % P
    v = vals.astype(BF)
    for f in range(nrow - 1):
        pk[a * nrow + f, col] = v[:, f]
    pk[a * nrow + (nrow - 1), col] = BF(1.0)
    return pk


def blockdiag_nb(w, A, sp):
    """No-bias block diag: [A*nf, A*sp]."""
    nf, k = w.shape
    bd = np.zeros((A * nf, A * sp), np.float32)
    for a in range(A):
        bd[a * nf:a * nf + nf, a * sp:a * sp + k] = w
    return bd.astype(BF)


def blockdiag_b(w, biasrow, A, sp):
    """Block diag with bias row: w [nf, sp*?], bias appended as last row."""
    nf, k = w.shape
    bd = np.zeros((A * (nf + 1), A * sp), np.float32)
    for a in range(A):
        bd[a * (nf + 1):a * (nf + 1) + nf, a * sp:a * sp + k] = w
        bd[a * (nf + 1) + nf, a * sp:a * sp + k] = biasrow
    return bd.astype(BF)


def pm(vals, nt):
    d = vals.shape[1]
    return np.ascontiguousarray(
        vals.reshape(nt, P, d).transpose(1, 0, 2).reshape(P, nt * d))


def l1_colperm_w(a_w, d_w):
    w = np.zeros((5, 12), np.float32)
    w[:, 0:2] = a_w
    for f in range(5):
        for h in range(2):
            w[:, 2 + 2 * f + h] = d_w[:, 5 * h + f]
    return w


def l1_colperm_b(a_b, d_b):
    b = np.zeros(12, np.float32)
    b[0:2] = a_b
    for f in range(5):
        for h in range(2):
            b[2 + 2 * f + h] = d_b[5 * h + f]
    return b


def d2_rowperm(w):
    out = np.zeros_like(w)
    for f in range(5):
        for h in range(2):
            out[2 * f + h] = w[5 * h + f]
    return out


def d2_colperm4(v):
    out = np.zeros_like(v)
    for f in range(2):
        for h in range(2):
            out[..., 2 * f + h] = v[..., 2 * h + f]
    return out


# ================================================================ device bits
def emit_project(nc, sb, ps, stg, bd, G, nb, N, grp, cswap):
    """Swapped-operand projection: per 128-col block,
    out[slot,:] = stg_block.T @ bd.  grp blocks share one PSUM tile."""
    ci = 0
    for g0 in range(0, nb, grp):
        gw = min(grp, nb - g0)
        pg = ps.tile([P, grp * N], f32, tag="pg")
        for j in range(gw):
            nc.tensor.matmul(out=pg[:, j * N:(j + 1) * N],
                             lhsT=stg[:, (g0 + j) * P:(g0 + j + 1) * P],
                             rhs=bd, start=True, stop=True)
        dst = G[:, g0 * N:(g0 + gw) * N]
        if cswap == -2 or ci % 3 == cswap:
            nc.vector.tensor_copy(out=dst, in_=pg[:, :gw * N])
        else:
            nc.scalar.copy(out=dst, in_=pg[:, :gw * N])
        ci += 1


def tree_sum_l(nc, e4, L, C, out4, first_pool=False):
    """Sum e4[:, :, l, :] over l with in-place pair adds; final into out4
    (shape [P, B, 1, C]). Optionally run the (big) first level on GpSimd."""
    n = L
    first = first_pool
    while n > 2:
        k = n // 2
        if first and k * B * C >= 1000:
            tt = nc.gpsimd.tensor_tensor
        else:
            tt = nc.vector.tensor_tensor
        first = False
        tt(out=e4[:, :, 0:k, :], in0=e4[:, :, 0:k, :],
           in1=e4[:, :, n - k:n, :], op=AL.add)
        n -= k
    if n == 2:
        nc.vector.tensor_tensor(out=out4, in0=e4[:, :, 0:1, :],
                                in1=e4[:, :, 1:2, :], op=AL.add)
    else:
        nc.vector.tensor_copy(out=out4, in_=e4[:, :, 0:1, :])


def emit_gat_supertile(nc, sb, sb3, G, L, st, nt, C, NH, nd, d_F,
                       ftab, attn, rstnE, denr, tree_pool=False):
    """GAT edge pipeline for one supertile, node-major.

    G keeps the raw projected source feats (so w = G*ex needs no bias or
    pad correction: pad slots have G == 0); e = G + fd lives in z.
    The main ops run in two b-halves so ACT/DVE pipeline within the tile."""
    t0 = st * B
    BL = B * L
    nrh = NH - nd
    G4 = G[:, 0:BL * C].rearrange("p (b l c) -> p b l c", b=B, l=L, c=C)
    fd_ap = ftab[:].rearrange("p (t c) -> p t c", t=nt, c=C)[:, t0:t0 + B, :]
    fdb = fd_ap.unsqueeze(2).broadcast_to([P, B, L, C])
    # z = G + fd, then prelu in place, then *attn in place
    z = sb.tile([P, BL * C], bf16, tag="z")
    z4 = z[:, 0:BL * C].rearrange("p (b l c) -> p b l c", b=B, l=L, c=C)
    nc.vector.tensor_tensor(out=z4, in0=G4, in1=fdb, op=AL.add)
    nc.scalar.activation(out=z[:, 0:BL * C], in_=z[:, 0:BL * C],
                         func=AF.Prelu, alpha=NEG_GAT)
    zq = z[:, 0:BL * C].rearrange("p (q c) -> p q c", q=BL, c=C)
    atb = attn[:].unsqueeze(1).broadcast_to([P, BL, C])
    nc.vector.tensor_tensor(out=zq, in0=zq, in1=atb, op=AL.mult)
    # scores: tree-sum over f for the d heads -> sd
    ex = sb3.tile([P, BL * NH], bf16, tag="ex")
    exq = ex[:, 0:BL * NH].rearrange("p (q h) -> p q h", q=BL, h=NH)
    if d_F == 5:
        s4 = sb3.tile([P, BL * 4], bf16, tag="s4")
        s4q = s4[:, 0:BL * 4].rearrange("p (q h) -> p q h", q=BL, h=4)
        nc.vector.tensor_tensor(out=s4q, in0=zq[:, :, nd:nd + 4],
                                in1=zq[:, :, nd + 4:nd + 8], op=AL.add)
        nc.vector.tensor_tensor(out=s4q[:, :, 0:2], in0=s4q[:, :, 0:2],
                                in1=s4q[:, :, 2:4], op=AL.add)
        nc.vector.tensor_tensor(out=s4q[:, :, 0:2], in0=s4q[:, :, 0:2],
                                in1=zq[:, :, nd + 8:nd + 10], op=AL.add)
        sdq = s4q[:, :, 0:2]
    else:  # d_F == 2
        s4 = sb3.tile([P, BL * 2], bf16, tag="s4")
        sdq = s4[:, 0:BL * 2].rearrange("p (q h) -> p q h", q=BL, h=2)
        nc.vector.tensor_tensor(out=sdq, in0=zq[:, :, nd:nd + 2],
                                in1=zq[:, :, nd + 2:nd + 4], op=AL.add)
    # ex = exp(scores)
    if nd:
        nc.scalar.activation(out=exq[:, :, 0:nd], in_=zq[:, :, 0:nd], func=AF.Exp)
    nc.scalar.activation(out=exq[:, :, nd:NH], in_=sdq, func=AF.Exp)
    # den_raw (incl pad slots): reduce over l
    ex4 = ex[:, 0:BL * NH].rearrange("p (b l h) -> p b h l", b=B, l=L, h=NH)
    den3 = denr[:, t0 * NH:(t0 + B) * NH].rearrange("p (b h) -> p b h", b=B, h=NH)
    nc.vector.tensor_reduce(out=den3, in_=ex4, axis=AX.X, op=AL.add)
    # w = G * ex  (in place in G; pad slots stay 0)
    exq4 = ex[:, 0:BL * NH].rearrange("p (b l h) -> p b l h", b=B, l=L, h=NH)
    if nd:
        nc.vector.tensor_tensor(out=G4[:, :, :, 0:nd], in0=G4[:, :, :, 0:nd],
                                in1=exq4[:, :, :, 0:nd], op=AL.mult)
    G5 = G4[:, :, :, nd:C].rearrange("p b l (f h) -> p b l f h", f=d_F, h=nrh)
    exd = exq4[:, :, :, nd:NH].unsqueeze(3).broadcast_to([P, B, L, d_F, nrh])
    nc.vector.tensor_tensor(out=G5, in0=G5, in1=exd, op=AL.mult)
    # rstnE = sum_l w  (tree adds)
    r4 = rstnE[:, t0 * C:(t0 + B) * C].rearrange("p (b u c) -> p b u c", b=B, u=1, c=C)
    tree_sum_l(nc, G4, L, C, r4, first_pool=tree_pool)


def emit_epilogue(nc, sb3, nt, ta, tb, C, NH, nd, d_F, ftab, ftabr, attn,
                  rstnE, denr, npad, h1o):
    """Per-node tail over tile range [ta, tb): pad-correct softmax denom,
    alpha-normalize, residual, elu."""
    nrh = NH - nd
    ng = tb - ta
    fts = ftab[:, ta * C:tb * C]
    dns = denr[:, ta * NH:tb * NH]
    rEs = rstnE[:, ta * C:tb * C]
    rEv = rEs.rearrange("p (t c) -> p t c", t=ng, c=C)
    # ---- pad-corrected denominator
    zp = sb3.tile([P, ng * C], bf16, tag="zp")
    zpf = zp[:, 0:ng * C]
    nc.vector.scalar_tensor_tensor(out=zpf, in0=fts, scalar=NEG_GAT,
                                   in1=fts, op0=AL.mult, op1=AL.max)
    zpv = zpf.rearrange("p (t c) -> p t c", t=ng, c=C)
    atb = attn[:].unsqueeze(1).broadcast_to([P, ng, C])
    nc.vector.tensor_tensor(out=zpv, in0=zpv, in1=atb, op=AL.mult)
    expp = sb3.tile([P, ng * NH], bf16, tag="expp")
    expf = expp[:, 0:ng * NH]
    expv = expf.rearrange("p (t h) -> p t h", t=ng, h=NH)
    if d_F == 5:
        sp4 = sb3.tile([P, ng * 4], bf16, tag="sp4")
        spv = sp4[:, 0:ng * 4].rearrange("p (t h) -> p t h", t=ng, h=4)
        nc.vector.tensor_tensor(out=spv, in0=zpv[:, :, nd:nd + 4],
                                in1=zpv[:, :, nd + 4:nd + 8], op=AL.add)
        nc.vector.tensor_tensor(out=spv[:, :, 0:2], in0=spv[:, :, 0:2],
                                in1=spv[:, :, 2:4], op=AL.add)
        nc.vector.tensor_tensor(out=spv[:, :, 0:2], in0=spv[:, :, 0:2],
                                in1=zpv[:, :, nd + 8:nd + 10], op=AL.add)
        spd = spv[:, :, 0:2]
    else:
        sp4 = sb3.tile([P, ng * 2], bf16, tag="sp4")
        spd = sp4[:, 0:ng * 2].rearrange("p (t h) -> p t h", t=ng, h=2)
        nc.vector.tensor_tensor(out=spd, in0=zpv[:, :, nd:nd + 2],
                                in1=zpv[:, :, nd + 2:nd + 4], op=AL.add)
    if nd:
        nc.scalar.activation(out=expv[:, :, 0:nd], in_=zpv[:, :, 0:nd], func=AF.Exp)
    nc.scalar.activation(out=expv[:, :, nd:NH], in_=spd, func=AF.Exp)
    nc.vector.tensor_tensor(out=expf, in0=expf, in1=npad[:, ta * NH:tb * NH],
                            op=AL.mult)
    nc.vector.tensor_tensor(out=dns, in0=dns, in1=expf, op=AL.subtract)
    nc.vector.tensor_scalar_max(out=dns, in0=dns, scalar1=1e-30)
    rec = sb3.tile([P, ng * NH], f32, tag="rec")
    recf = rec[:, 0:ng * NH]
    nc.vector.reciprocal_approx_fast(out=recf, in_=dns)
    # ---- alpha-normalize (in place in rstnE)
    rcv = recf.rearrange("p (t h) -> p t h", t=ng, h=NH)
    if nd:
        nc.vector.tensor_tensor(out=rEv[:, :, 0:nd], in0=rEv[:, :, 0:nd],
                                in1=rcv[:, :, 0:nd], op=AL.mult)
    rE5 = rEv[:, :, nd:C].rearrange("p t (f h) -> p t f h", f=d_F, h=nrh)
    rcb = rcv[:, :, nd:NH].unsqueeze(2).broadcast_to([P, ng, d_F, nrh])
    nc.vector.tensor_tensor(out=rE5, in0=rE5, in1=rcb, op=AL.mult)
    # ---- + residual; elu -> h1o (bf16)
    nc.vector.tensor_tensor(out=rEs, in0=rEs, in1=ftabr[:, ta * C:tb * C],
                            op=AL.add)
    tmin = sb3.tile([P, ng * C], bf16, tag="tmin")
    tmf = tmin[:, 0:ng * C]
    nc.vector.tensor_scalar_min(out=tmf, in0=rEs, scalar1=0.0)
    epx = sb3.tile([P, ng * C], bf16, tag="epx")
    epf = epx[:, 0:ng * C]
    nc.scalar.activation(out=epf, in_=tmf, func=AF.Exp)
    nc.vector.tensor_scalar_max(out=rEs, in0=rEs, scalar1=0.0)
    nc.vector.scalar_tensor_tensor(out=h1o[:, ta * C:tb * C], in0=epf,
                                   scalar=-1.0, in1=rEs, op0=AL.add, op1=AL.add)


# =============================================================== launches
def build_launch1(nst, Ls, nblks, totc, nt):
    nchunk = nt // 8
    col_off = (np.concatenate([[0], np.cumsum(nblks)]) * P).astype(np.int64)
    nc = bacc.Bacc("TRN2", target_bir_lowering=False, debug=False, num_devices=NCORE)
    d_pk = nc.dram_tensor("x5e", [A1 * 5, totc], bf16, kind="ExternalInput")
    d_lpk = nc.dram_tensor("x5l", [48, nchunk * P], bf16, kind="ExternalInput")
    d_np = nc.dram_tensor("npad4", [P, nt * 4], bf16, kind="ExternalInput")
    d_at = nc.dram_tensor("attn12", [P, 12], bf16, kind="ExternalInput")
    d_bs = nc.dram_tensor("bd_src", [A1 * 5, A1 * 12], bf16, kind="ExternalInput")
    d_bl = nc.dram_tensor("bd_loc", [48, 192], bf16, kind="ExternalInput")
    d_h1o = nc.dram_tensor("h1o", [P, nt * 12], bf16, kind="ExternalOutput")
    with TileContext(nc) as tc:
        with tc.tile_pool(name="res", bufs=1) as res, \
             tc.tile_pool(name="sb", bufs=3) as sb, \
             tc.tile_pool(name="sb3", bufs=3) as sb3, \
             tc.tile_pool(name="ps", bufs=4, space="PSUM") as ps, \
             tc.tile_pool(name="psl", bufs=2, space="PSUM") as psl:
            attn = res.tile([P, 12], bf16)
            nc.sync.dma_start(out=attn[:], in_=d_at[:, :])
            npad = res.tile([P, nt * 4], bf16)
            nc.sync.dma_start(out=npad[:], in_=d_np[:, :])
            bds = res.tile([A1 * 5, A1 * 12], bf16, tag="bds")
            nc.sync.dma_start(out=bds[:], in_=d_bs[:, :])
            bdl = res.tile([48, 192], bf16, tag="bdl")
            nc.sync.dma_start(out=bdl[:], in_=d_bl[:, :])
            ftab = res.tile([P, nt * 12], bf16)
            ftabr = res.tile([P, nt * 12], bf16)
            rstnE = res.tile([P, nt * 12], bf16)
            denr = res.tile([P, nt * 4], f32)
            h1o = res.tile([P, nt * 12], bf16)
            # local projections (fd' and res', biases folded in)
            for ch in range(nchunk):
                stg = sb.tile([48, P], bf16, tag="lstg")
                nc.sync.dma_start(out=stg[:], in_=d_lpk[:, ch * P:(ch + 1) * P])
                pmm = psl.tile([P, 192], f32, tag="lmm")
                nc.tensor.matmul(out=pmm[:], lhsT=stg[:], rhs=bdl[:],
                                 start=True, stop=True)
                pv = pmm[:].rearrange("p (a k) -> p a k", a=8, k=24)
                fsl = ftab[:].rearrange("p (t c) -> p t c", t=nt, c=12)[:, ch * 8:(ch + 1) * 8, :]
                nc.scalar.copy(out=fsl, in_=pv[:, :, 0:12])
                rsl = ftabr[:].rearrange("p (t c) -> p t c", t=nt, c=12)[:, ch * 8:(ch + 1) * 8, :]
                nc.scalar.copy(out=rsl, in_=pv[:, :, 12:24])
            # supertiles: project one supertile ahead of the GAT pipeline so
            # the ACT-queue copies for st+1 precede st's prelu/exps
            def project1(st):
                nb = nblks[st]
                c0 = int(col_off[st])
                stg = sb.tile([A1 * 5, nb * P], bf16, tag="estg")
                nc.sync.dma_start(out=stg[:], in_=d_pk[:, c0:c0 + nb * P])
                G = sb.tile([P, nb * A1 * 12], bf16, tag="G")
                emit_project(nc, sb, ps, stg[:], bds[:], G[:], nb, A1 * 12, 2, -1)
                return G
            Gs = {0: project1(0)}
            for st in range(nst):
                if st + 1 < nst:
                    Gs[st + 1] = project1(st + 1)
                emit_gat_supertile(nc, sb, sb3, Gs.pop(st), Ls[st], st, nt,
                                   12, 4, 2, 5, ftab, attn, rstnE, denr)
            emit_epilogue(nc, res, nt, 0, nt, 12, 4, 2, 5, ftab, ftabr, attn,
                          rstnE, denr, npad, h1o)
            nc.sync.dma_start(out=d_h1o[:, :], in_=h1o[:])
    nc.compile()
    return nc


def build_launch2(nst, Ls, nblks, totc, nt):
    nchunk = nt // 8
    nmc = nt * P // 512
    col_off = (np.concatenate([[0], np.cumsum(nblks)]) * P).astype(np.int64)
    nc = bacc.Bacc("TRN2", target_bir_lowering=False, debug=False, num_devices=NCORE)
    d_pk = nc.dram_tensor("hde", [A2 * 10, totc], bf16, kind="ExternalInput")
    d_lpk = nc.dram_tensor("h1l", [88, nchunk * P], bf16, kind="ExternalInput")
    d_np = nc.dram_tensor("npad2", [P, nt * 2], bf16, kind="ExternalInput")
    d_at = nc.dram_tensor("attn4", [P, 4], bf16, kind="ExternalInput")
    d_b2e = nc.dram_tensor("bd2e", [A2 * 10, A2 * 4], bf16, kind="ExternalInput")
    d_b2l = nc.dram_tensor("bd2l", [88, 64], bf16, kind="ExternalInput")
    d_hx = nc.dram_tensor("hx", [P, nt * 10], bf16, kind="ExternalInput")
    d_w1 = nc.dram_tensor("w1", [14, 196], bf16, kind="ExternalInput")
    d_w2 = nc.dram_tensor("w2", [196, 196], bf16, kind="ExternalInput")
    d_w3 = nc.dram_tensor("w3", [196, 14], bf16, kind="ExternalInput")
    d_w4 = nc.dram_tensor("w4n", [14, 1], bf16, kind="ExternalInput")
    d_b1 = nc.dram_tensor("b1", [196], f32, kind="ExternalInput")
    d_b2 = nc.dram_tensor("b2", [196], f32, kind="ExternalInput")
    d_b3 = nc.dram_tensor("b3", [14], f32, kind="ExternalInput")
    d_b4 = nc.dram_tensor("b4n", [1], f32, kind="ExternalInput")
    d_out = nc.dram_tensor("out", [nmc, 512], f32, kind="ExternalOutput")
    with TileContext(nc) as tc:
        with tc.tile_pool(name="res", bufs=1) as res, \
             tc.tile_pool(name="sb", bufs=3) as sb, \
             tc.tile_pool(name="sb3", bufs=2) as sb3, \
             tc.tile_pool(name="ps", bufs=1, space="PSUM") as ps, \
             tc.tile_pool(name="psl", bufs=3, space="PSUM") as psl, \
             tc.tile_pool(name="pso", bufs=1, space="PSUM") as pso:
            ident = res.tile([P, P], bf16)
            make_identity(nc, ident[:])
            attn = res.tile([P, 4], bf16)
            nc.sync.dma_start(out=attn[:], in_=d_at[:, :])
            npad = res.tile([P, nt * 2], bf16)
            nc.sync.dma_start(out=npad[:], in_=d_np[:, :])
            b2e = res.tile([A2 * 10, A2 * 4], bf16, tag="b2e")
            nc.sync.dma_start(out=b2e[:], in_=d_b2e[:, :])
            b2l = res.tile([88, 64], bf16, tag="b2l")
            nc.sync.dma_start(out=b2l[:], in_=d_b2l[:, :])
            hx = res.tile([P, nt * 10], bf16)
            nc.sync.dma_start(out=hx[:], in_=d_hx[:, :])
            ftab2 = res.tile([P, nt * 4], bf16)
            ftab2r = res.tile([P, nt * 4], bf16)
            rstnE2 = res.tile([P, nt * 4], bf16)
            denr2 = res.tile([P, nt * 2], f32)
            h2o = res.tile([P, nt * 4], bf16)
            w1 = res.tile([14, 196], bf16, tag="w1")
            nc.sync.dma_start(out=w1[:], in_=d_w1[:, :])
            w2a = res.tile([P, 196], bf16, tag="w2a")
            nc.sync.dma_start(out=w2a[:], in_=d_w2[0:128, :])
            w2b = res.tile([68, 196], bf16, tag="w2b")
            nc.sync.dma_start(out=w2b[:], in_=d_w2[128:196, :])
            w3a = res.tile([P, 14], bf16, tag="w3a")
            nc.sync.dma_start(out=w3a[:], in_=d_w3[0:128, :])
            w3b = res.tile([68, 14], bf16, tag="w3b")
            nc.sync.dma_start(out=w3b[:], in_=d_w3[128:196, :])
            w4 = res.tile([14, 1], bf16, tag="w4")
            nc.sync.dma_start(out=w4[:], in_=d_w4[:, :])
            b1ca = res.tile([P, 1], f32, tag="b1ca")
            nc.sync.dma_start(out=b1ca[:], in_=d_b1[0:128, None])
            b1cb = res.tile([68, 1], f32, tag="b1cb")
            nc.sync.dma_start(out=b1cb[:], in_=d_b1[128:196, None])
            b2ca = res.tile([P, 1], f32, tag="b2ca")
            nc.sync.dma_start(out=b2ca[:], in_=d_b2[0:128, None])
            b2cb = res.tile([68, 1], f32, tag="b2cb")
            nc.sync.dma_start(out=b2cb[:], in_=d_b2[128:196, None])
            b3c = res.tile([14, 1], f32, tag="b3c")
            nc.sync.dma_start(out=b3c[:], in_=d_b3[:, None])
            b4c = res.tile([1, 1], f32, tag="b4c")
            nc.sync.dma_start(out=b4c[:], in_=d_b4[:, None])
            # local projections (fd2' and res2', biases + bs2 folded)
            for ch in range(nchunk):
                stg = sb.tile([88, P], bf16, tag="lstg")
                nc.sync.dma_start(out=stg[:], in_=d_lpk[:, ch * P:(ch + 1) * P])
                pmm = ps.tile([P, 64], f32, tag="pg")
                nc.tensor.matmul(out=pmm[:], lhsT=stg[:], rhs=b2l[:],
                                 start=True, stop=True)
                pv = pmm[:].rearrange("p (a k) -> p a k", a=8, k=8)
                fsl = ftab2[:].rearrange("p (t c) -> p t c", t=nt, c=4)[:, ch * 8:(ch + 1) * 8, :]
                nc.vector.tensor_copy(out=fsl, in_=pv[:, :, 0:4])
                rsl = ftab2r[:].rearrange("p (t c) -> p t c", t=nt, c=4)[:, ch * 8:(ch + 1) * 8, :]
                nc.vector.tensor_copy(out=rsl, in_=pv[:, :, 4:8])
            # supertiles (GAT layer d2)
            for st in range(nst):
                L = Ls[st]
                nb = nblks[st]
                c0 = int(col_off[st])
                stg = sb.tile([A2 * 10, nb * P], bf16, tag="estg")
                nc.sync.dma_start(out=stg[:], in_=d_pk[:, c0:c0 + nb * P])
                G = sb.tile([P, nb * A2 * 4], bf16, tag="G")
                emit_project(nc, sb, ps, stg[:], b2e[:], G[:], nb, A2 * 4, 10, -2)
                emit_gat_supertile(nc, sb, sb3, G, L, st, nt, 4, 2, 0, 2,
                                   ftab2, attn, rstnE2, denr2)
                # grouped epilogue every 4 supertiles so MLP chunks unblock
                if st % 4 == 3 or st == nst - 1:
                    ta = (st // 4) * 4 * B
                    emit_epilogue(nc, sb3, nt, ta, (st + 1) * B, 4, 2, 0, 2,
                                  ftab2, ftab2r, attn, rstnE2, denr2, npad, h2o)
            # MLP over pairs of 512-node chunks (interleaved so PE stays busy
            # while ACT runs the other chunk's activation)
            for mc0 in range(0, nmc, 3):
                pair = [m for m in (mc0, mc0 + 1, mc0 + 2) if m < nmc]
                r0s, p1s, r1s, p2s, r2s, p3s, r3s, pos, egs = [], [], [], [], [], [], [], [], []
                for mc in pair:
                    t0 = mc * 4
                    m14 = sb.tile([P, 4 * 32], bf16, tag="m14")
                    m143 = m14[:].rearrange("p (t c) -> p t c", t=4, c=32)
                    hxs = hx[:].rearrange("p (t c) -> p t c", t=nt, c=10)[:, t0:t0 + 4, :]
                    nc.vector.tensor_copy(out=m143[:, :, 0:10], in_=hxs)
                    h2s = h2o[:].rearrange("p (t c) -> p t c", t=nt, c=4)[:, t0:t0 + 4, :]
                    nc.vector.tensor_copy(out=m143[:, :, 10:14], in_=h2s)
                    pt = pso.tile([P, P], bf16, tag="ltt")
                    nc.tensor.transpose(out=pt[:], in_=m14[:], identity=ident[:])
                    ptc = sb.tile([P, P], bf16, tag="ptc")
                    nc.vector.tensor_copy(out=ptc[:], in_=pt[:])
                    r0 = sb.tile([14, 512], bf16, tag="r0")
                    for b in range(4):
                        nc.vector.tensor_copy(out=r0[:, b * P:(b + 1) * P],
                                              in_=ptc[b * 32:b * 32 + 14, :])
                    r0s.append(r0)
                for r0 in r0s:
                    p1a = psl.tile([P, 512], f32, tag="pA")
                    nc.tensor.matmul(out=p1a[:], lhsT=w1[:, 0:128], rhs=r0[:], start=True, stop=True)
                    p1b = psl.tile([68, 512], f32, tag="pB")
                    nc.tensor.matmul(out=p1b[:], lhsT=w1[:, 128:196], rhs=r0[:], start=True, stop=True)
                    p1s.append((p1a, p1b))
                for p1a, p1b in p1s:
                    r1a = sb.tile([P, 512], bf16, tag="r1a")
                    nc.scalar.activation(out=r1a[:], in_=p1a[:], func=AF.Prelu,
                                         alpha=NEG_MLP, bias=b1ca[:])
                    r1b = sb.tile([68, 512], bf16, tag="r1b")
                    nc.scalar.activation(out=r1b[:], in_=p1b[:], func=AF.Prelu,
                                         alpha=NEG_MLP, bias=b1cb[:])
                    r1s.append((r1a, r1b))
                for r1a, r1b in r1s:
                    p2a = psl.tile([P, 512], f32, tag="pA")
                    nc.tensor.matmul(out=p2a[:], lhsT=w2a[:, 0:128], rhs=r1a[:], start=True, stop=False)
                    nc.tensor.matmul(out=p2a[:], lhsT=w2b[:, 0:128], rhs=r1b[:], start=False, stop=True)
                    p2b = psl.tile([68, 512], f32, tag="pB")
                    nc.tensor.matmul(out=p2b[:], lhsT=w2a[:, 128:196], rhs=r1a[:], start=True, stop=False)
                    nc.tensor.matmul(out=p2b[:], lhsT=w2b[:, 128:196], rhs=r1b[:], start=False, stop=True)
                    p2s.append((p2a, p2b))
                for p2a, p2b in p2s:
                    r2a = sb.tile([P, 512], bf16, tag="r2a")
                    nc.scalar.activation(out=r2a[:], in_=p2a[:], func=AF.Prelu,
                                         alpha=NEG_MLP, bias=b2ca[:])
                    r2b = sb.tile([68, 512], bf16, tag="r2b")
                    nc.scalar.activation(out=r2b[:], in_=p2b[:], func=AF.Prelu,
                                         alpha=NEG_MLP, bias=b2cb[:])
                    r2s.append((r2a, r2b))
                for r2a, r2b in r2s:
                    p3 = psl.tile([14, 512], f32, tag="pA")
                    nc.tensor.matmul(out=p3[:], lhsT=w3a[:], rhs=r2a[:], start=True, stop=False)
                    nc.tensor.matmul(out=p3[:], lhsT=w3b[:], rhs=r2b[:], start=False, stop=True)
                    p3s.append(p3)
                for p3 in p3s:
                    r3 = sb.tile([14, 512], bf16, tag="r3")
                    nc.scalar.activation(out=r3[:], in_=p3[:], func=AF.Prelu,
                                         alpha=NEG_MLP, bias=b3c[:])
                    r3s.append(r3)
                for r3 in r3s:
                    po = psl.tile([1, 512], f32, tag="pB")
                    nc.tensor.matmul(out=po[:], lhsT=w4[:], rhs=r3[:], start=True, stop=True)
                    pos.append(po)
                # sigmoid(x) = 1/(1+exp(-x)); po = -x already (w4, b4 negated)
                nq = len(pair)
                eg4 = sb.tile([1, 3 * 512], f32, tag="eg4")
                for k, po in enumerate(pos):
                    nc.scalar.activation(out=eg4[:, k * 512:(k + 1) * 512],
                                         in_=po[:], func=AF.Exp, bias=b4c[:])
                sl = eg4[:, 0:nq * 512]
                nc.vector.tensor_scalar_add(out=sl, in0=sl, scalar1=1.0)
                sg4 = sb.tile([1, 3 * 512], f32, tag="sg4")
                nc.vector.reciprocal_approx_fast(out=sg4[:, 0:nq * 512], in_=sl)
                nc.sync.dma_start(out=d_out[mc0:mc0 + nq, :], in_=sg4[:, 0:nq * 512])
    nc.compile()
    return nc


_cache = {}


def kernel(**inputs):
    x = np.asarray(inputs['x'], np.float32)
    src = np.asarray(inputs['src'], np.int32)
    dst = np.asarray(inputs['dst'], np.int32)
    n = x.shape[0]

    scheds, nst, Ls = build_schedule(dst, n)
    nt = scheds[0]['nt']
    nloc = scheds[0]['nloc']
    nblk1 = [-(-(B * L) // A1) for L in Ls]
    nblk2 = [-(-(B * L) // A2) for L in Ls]
    totc1 = int(sum(nblk1)) * P
    totc2 = int(sum(nblk2)) * P

    # ---- weights (host-side layout/permutation + bias folding)
    wsrc1 = l1_colperm_w(np.asarray(inputs['a1_Wsrc']), np.asarray(inputs['d1_Wsrc']))
    bsrc1 = l1_colperm_b(np.asarray(inputs['a1_bsrc']), np.asarray(inputs['d1_bsrc']))
    bd_src = blockdiag_nb(wsrc1, A1, 12)
    wloc = np.zeros((5, 24), np.float32)
    wloc[:, 0:12] = l1_colperm_w(np.asarray(inputs['a1_Wdst']), np.asarray(inputs['d1_Wdst']))
    wloc[:, 12:24] = l1_colperm_w(np.asarray(inputs['a1_Wres']), np.asarray(inputs['d1_Wres']))
    bloc = np.zeros(24, np.float32)
    bloc[0:12] = l1_colperm_b(np.asarray(inputs['a1_bdst']), np.asarray(inputs['d1_bdst'])) + bsrc1
    bloc[12:24] = l1_colperm_b(np.asarray(inputs['a1_bias']), np.asarray(inputs['d1_bias'])) + bsrc1
    bd_loc = blockdiag_b(wloc, bloc, 8, 24)
    attn12 = np.zeros(12, np.float32)
    attn12[0:2] = np.asarray(inputs['a1_attn'])[:, 0]
    for f in range(5):
        for h in range(2):
            attn12[2 + 2 * f + h] = np.asarray(inputs['d1_attn'])[h, f]
    attn12_t = np.tile(attn12.astype(BF), (P, 1))

    ws2 = d2_rowperm(d2_colperm4(np.asarray(inputs['d2_Wsrc'], np.float32)))
    bs2 = d2_colperm4(np.asarray(inputs['d2_bsrc'], np.float32))
    wd2 = d2_rowperm(d2_colperm4(np.asarray(inputs['d2_Wdst'], np.float32)))
    bdst2 = d2_colperm4(np.asarray(inputs['d2_bdst'], np.float32))
    wr2 = d2_rowperm(d2_colperm4(np.asarray(inputs['d2_Wres'], np.float32)))
    bias2 = d2_colperm4(np.asarray(inputs['d2_bias'], np.float32))
    bd2e = blockdiag_nb(ws2, A2, 4)
    wloc2 = np.concatenate([wd2, wr2], axis=1)
    bloc2 = np.concatenate([bdst2 + bs2, bias2 + bs2])
    bd2l = blockdiag_b(wloc2, bloc2, 8, 8)
    attn4 = np.zeros(4, np.float32)
    for f in range(2):
        for h in range(2):
            attn4[2 * f + h] = np.asarray(inputs['d2_attn'])[h, f]
    attn4_t = np.tile(attn4.astype(BF), (P, 1))

    w1p = np.asarray(inputs['W1'], np.float32).copy()
    for f in range(2):
        for h in range(2):
            w1p[2 + 2 * f + h] = np.asarray(inputs['W1'])[2 + 2 * h + f]
    w1p = w1p[[0, 1, 6, 7, 8, 9, 10, 11, 12, 13, 2, 3, 4, 5], :]

    key = (n, len(src), nst, tuple(Ls))
    if key not in _cache:
        _cache[key] = (build_launch1(nst, Ls, nblk1, totc1, nt),
                       build_launch2(nst, Ls, nblk2, totc2, nt))
    nc1, nc2 = _cache[key]

    in1, core_meta = [], []
    for s in scheds:
        eo1, a1e, col1, _, _ = edge_slot_cols(s, Ls, A1)
        core_meta.append((s, eo1))
        x5e = pack_edges(x[src[s['em']], :5], eo1, a1e, col1, totc1, A1)
        orig = s['order']
        valid = orig < nloc
        xl = np.zeros((nt * P, 5), np.float32)
        xl[valid] = x[s['core'] * nloc + orig[valid], :5]
        in1.append(dict(x5e=x5e, x5l=pack_local(xl, 6, nt),
                        npad4=make_npad(s, Ls, 4),
                        attn12=attn12_t, bd_src=bd_src, bd_loc=bd_loc))
    r1 = run_bass_kernel_spmd(nc1, in1, core_ids=list(range(NCORE)))
    t1 = r1.exec_time_ns or 0

    hdef_g = np.zeros((n, 10), np.float32)
    hatt_all, h1_all = [], []
    for ci, s in enumerate(scheds):
        h1 = np.asarray(r1.results[ci]['h1o'], np.float32).reshape(P, nt, 12).transpose(1, 0, 2).reshape(nt * P, 12)
        h1_all.append(h1)
        orig = s['order']
        valid = orig < nloc
        hdef_g[s['core'] * nloc + orig[valid]] = h1[valid][:, 2:12]
        hatt_all.append(h1[:, 0:2])

    in2 = []
    for ci, (s, _) in enumerate(core_meta):
        eo2, a2e, col2, _, _ = edge_slot_cols(s, Ls, A2)
        hde = pack_edges(hdef_g[src[s['em']]], eo2, a2e, col2, totc2, A2)
        orig = s['order']
        valid = orig < nloc
        xl8 = np.zeros((nt * P, 8), np.float32)
        xl8[valid] = x[s['core'] * nloc + orig[valid], :]
        in2.append(dict(hde=hde, h1l=pack_local(h1_all[ci][:, 2:12], 11, nt),
                        npad2=make_npad(s, Ls, 2), attn4=attn4_t,
                        bd2e=bd2e, bd2l=bd2l,
                        hx=pm(np.concatenate([hatt_all[ci], xl8], axis=1), nt).astype(BF),
                        w1=w1p.astype(BF),
                        w2=np.asarray(inputs['W2'], np.float32).astype(BF),
                        w3=np.asarray(inputs['W3'], np.float32).astype(BF),
                        w4n=(-np.asarray(inputs['W4'], np.float32)).astype(BF),
                        b1=np.asarray(inputs['b1'], np.float32),
                        b2=np.asarray(inputs['b2'], np.float32),
                        b3=np.asarray(inputs['b3'], np.float32),
                        b4n=-np.asarray(inputs['b4'], np.float32)))
    r2 = run_bass_kernel_spmd(nc2, in2, core_ids=list(range(NCORE)))
    t2 = r2.exec_time_ns or 0

    out = np.zeros((n, 1), np.float32)
    for ci, s in enumerate(scheds):
        y = r2.results[ci]['out'].reshape(nt * P)
        orig = s['order']
        valid = orig < nloc
        out[s['core'] * nloc + orig[valid], 0] = y[valid]
    kernel.last_exec_ns = t1 + t2
    kernel.last_t12 = (t1, t2)
    kernel.last_results = (r1, r2)
    return out
